# revision 1
# baseline (speedup 1.0000x reference)
"""Trainium2 Bass kernel for GNN NodeBlock (segment_sum + MLP), 8-core SPMD.

Strategy (node-sharded, two-path aggregation, transpose-free):
  - Shard the 100k nodes across 8 cores (12500 each). Host sorts edges by
    receiver.
  - Dense path: each node gets 16 padded edge slots (~90% of edges). The
    host lays each pair of slot-planes out as a [128, 512] tile whose
    partition axis is (slot-pair-member, hi/lo, feature) and free axis is
    the 512 nodes of a supergroup. A constant stationary [I32;I32;I32;I32]
    then makes each matmul compute psum[f, n] += hi+lo of both slots — the
    PSUM accumulates the feature-major aggregate directly (no one-hot, no
    transpose, no weight reloads).
  - Overflow path: edges beyond a node's 16th go through a one-hot matmul
    (is_equal vs iota on DVE): psum[:, window] += edges_hi^T @ onehot and
    edges_lo^T @ onehot.
  - The [32, 512] aggregate is drained once per supergroup as bf16 hi
    (ACT cast-copy) + bf16 lo (DVE subtract), duplicated into a K-stacked
    [hi; hi; lo] layout by SBUF-to-SBUF DMA, and consumed by K-stacked bf16
    MLP matmuls (exact to ~2^-16) with weight hi/lo splits baked in on the
    host. The globals term is folded into b1 on the host.
  - No collectives: cores own disjoint node ranges; host concatenates.
"""

import os

import numpy as np
import ml_dtypes

import concourse.bacc as bacc
import concourse.bass as bass
import concourse.mybir as mybir
import concourse.tile as tile
from concourse.bass_utils import run_bass_kernel_spmd

BF16 = ml_dtypes.bfloat16

N_NODES = 100000
N_CORES = 8
NPC = N_NODES // N_CORES  # 12500 nodes per core
P = 128
SG = 512  # nodes per supergroup (4 windows of 128)
NSG = -(-NPC // SG)  # 25 supergroups per core
WPS = SG // P  # 4 windows per supergroup
G = NSG * WPS  # 100 windows incl. dummies (98 real)
KD = 16  # dense slots per node
QD = KD // 2  # slot pairs -> dense matmuls per supergroup
D = 32

_prog_cache = {}


def _split_hi_lo(x):
    hi = x.astype(BF16)
    lo = (x - hi.astype(np.float32)).astype(BF16)
    return hi, lo


def _host_prep(node_attr, edge_index, edge_attr, global_attr, W1, b1, W2, b2):
    E = edge_attr.shape[0]
    r = np.ascontiguousarray(edge_index[1]).astype(np.int64)

    order = np.argsort(r, kind="stable")
    r_s = r[order]
    deg = np.bincount(r, minlength=N_NODES)
    starts = np.zeros(N_NODES, dtype=np.int64)
    np.cumsum(deg[:-1], out=starts[1:])
    k = np.arange(E, dtype=np.int64) - starts[r_s]  # rank within node

    ea = np.ascontiguousarray(edge_attr, dtype=np.float32)
    hi, lo = _split_hi_lo(ea)
    hilo = np.concatenate([hi, lo], axis=1)[order]  # (E, 64) in sorted order

    core = r_s // NPC
    local = r_s - core * NPC
    sg = local // SG
    n512 = local % SG
    j = n512 // P
    w_in = (local % P).astype(np.float32)

    dense = k < KD
    # (c, sg, q, sp, n, chan) -> partitions (sp, chan), free (q, n)
    TA = np.zeros((N_CORES, NSG, QD, 2, SG, 2 * D), dtype=BF16)
    TA[core[dense], sg[dense], k[dense] // 2, k[dense] % 2, n512[dense]] = hilo[dense]
    arr_A = np.ascontiguousarray(TA.transpose(0, 1, 3, 5, 2, 4)).reshape(
        N_CORES, NSG, P, QD * SG
    )

    # overflow: bucket per (core, window); windows 98,99 are dummies
    ov = ~dense
    wkey = (core * G + local // P).astype(np.int64)[ov]
    cnt = np.bincount(wkey, minlength=N_CORES * G)
    m_l = max(1, int(-(-cnt.max() // P)))
    starts2 = np.zeros(N_CORES * G, dtype=np.int64)
    np.cumsum(cnt[:-1], out=starts2[1:])
    seq = np.zeros(E, dtype=np.int64)
    seq[ov] = np.arange(int(ov.sum()), dtype=np.int64)
    slot2 = seq[ov] - starts2[wkey]
    t2 = slot2 // P
    p2 = slot2 % P

    arr_B = np.zeros((N_CORES, NSG, P, WPS, m_l, 2 * D), dtype=BF16)
    arr_B[core[ov], sg[ov], p2, j[ov], t2] = hilo[ov]

    widx = np.full((N_CORES * G, m_l * P), -1.0, dtype=np.float32)
    widx[wkey, slot2] = w_in[ov]
    # (c, w=sg*4+jj, t, p) -> (c, p, sg, jj, t)
    idx_arr = np.ascontiguousarray(
        widx.reshape(N_CORES, NSG, WPS, m_l, P).transpose(0, 4, 1, 2, 3)
    ).reshape(N_CORES, P, NSG * WPS * m_l).astype(BF16)

    edges_in = np.concatenate(
        [
            arr_A,
            arr_B.reshape(N_CORES, NSG, P, WPS * m_l * 2 * D),
        ],
        axis=3,
    )

    # stationary for the dense path: [I32; I32; I32; I32]
    ident4 = np.ascontiguousarray(np.tile(np.eye(D, dtype=BF16), (4, 1)))

    nodeT = np.ascontiguousarray(
        node_attr.astype(np.float32).reshape(N_CORES, NPC, D).transpose(0, 2, 1)
    )
    nhi, nlo = _split_hi_lo(nodeT)
    node_stack = np.concatenate([nhi, nhi, nlo], axis=1)  # (8, 96, NPC)

    g0 = global_attr.astype(np.float32).reshape(1, D)
    W1 = W1.astype(np.float32)
    b1p = (b1.astype(np.float32) + (g0 @ W1[2 * D :]).reshape(-1)).reshape(D, 1)
    w1a_hi, w1a_lo = _split_hi_lo(W1[:D])
    w1b_hi, w1b_lo = _split_hi_lo(W1[D : 2 * D])
    w1n = np.ascontiguousarray(np.concatenate([w1a_hi, w1a_lo, w1a_hi], axis=0))
    w1g = np.ascontiguousarray(np.concatenate([w1b_hi, w1b_lo, w1b_hi], axis=0))
    w2_hi, w2_lo = _split_hi_lo(W2.astype(np.float32))
    w2_hi = np.ascontiguousarray(w2_hi)
    w2_lo = np.ascontiguousarray(w2_lo)
    b2 = b2.astype(np.float32).reshape(D, 1)

    in_maps = []
    for c in range(N_CORES):
        in_maps.append(
            {
                "edges": edges_in[c],
                "idx": idx_arr[c],
                "ident4": ident4,
                "nodeS": node_stack[c],
                "w1n": w1n,
                "w1g": w1g,
                "w2h": w2_hi,
                "w2l": w2_lo,
                "b1p": b1p,
                "b2": b2,
            }
        )
    return in_maps, m_l


def _build_program(m_l):
    if m_l in _prog_cache:
        return _prog_cache[m_l]

    f32 = mybir.dt.float32
    bf16 = mybir.dt.bfloat16
    nc = bacc.Bacc(
        "TRN2", target_bir_lowering=False, debug=False, num_devices=N_CORES
    )

    A_ELEMS = QD * SG  # 4096
    B_ELEMS = WPS * m_l * 2 * D
    NT = WPS * m_l  # overflow tiles per supergroup

    edges_d = nc.dram_tensor(
        "edges", [NSG, P, A_ELEMS + B_ELEMS], bf16, kind="ExternalInput"
    )
    idx_d = nc.dram_tensor("idx", [P, NSG * NT], bf16, kind="ExternalInput")
    ident4_d = nc.dram_tensor("ident4", [P, D], bf16, kind="ExternalInput")
    nodeS_d = nc.dram_tensor("nodeS", [3 * D, NPC], bf16, kind="ExternalInput")
    w1n_d = nc.dram_tensor("w1n", [3 * D, D], bf16, kind="ExternalInput")
    w1g_d = nc.dram_tensor("w1g", [3 * D, D], bf16, kind="ExternalInput")
    w2h_d = nc.dram_tensor("w2h", [D, D], bf16, kind="ExternalInput")
    w2l_d = nc.dram_tensor("w2l", [D, D], bf16, kind="ExternalInput")
    b1p_d = nc.dram_tensor("b1p", [D, 1], f32, kind="ExternalInput")
    b2_d = nc.dram_tensor("b2", [D, 1], f32, kind="ExternalInput")
    outT_d = nc.dram_tensor("outT", [D, NPC], f32, kind="ExternalOutput")

    with tile.TileContext(nc) as tc:
        with (
            tc.tile_pool(name="const", bufs=1) as cpool,
            tc.tile_pool(name="edges", bufs=3) as epool,
            tc.tile_pool(name="oh", bufs=3) as opool,
            tc.tile_pool(name="mlp", bufs=2) as mpool,
            tc.tile_pool(name="psA", bufs=4, space="PSUM") as pspool,
            tc.tile_pool(name="psM", bufs=2, space="PSUM") as pmpool,
        ):
            # constants
            iota32 = cpool.tile([P, NT, P], mybir.dt.int32)
            nc.gpsimd.iota(
                iota32[:], pattern=[[0, NT], [1, P]], base=0, channel_multiplier=0
            )
            iotab = cpool.tile([P, NT, P], bf16)
            nc.vector.tensor_copy(out=iotab[:], in_=iota32[:])

            ident4_sb = cpool.tile([P, D], bf16)
            nc.sync.dma_start(out=ident4_sb[:], in_=ident4_d.ap())
            idx_all = cpool.tile([P, NSG * NT], bf16)
            nc.sync.dma_start(out=idx_all[:], in_=idx_d.ap())
            nodeS_sb = cpool.tile([3 * D, NPC], bf16)
            nc.sync.dma_start(out=nodeS_sb[:], in_=nodeS_d.ap())
            w1n_sb = cpool.tile([3 * D, D], bf16)
            nc.sync.dma_start(out=w1n_sb[:], in_=w1n_d.ap())
            w1g_sb = cpool.tile([3 * D, D], bf16)
            nc.sync.dma_start(out=w1g_sb[:], in_=w1g_d.ap())
            w2h_sb = cpool.tile([D, D], bf16)
            nc.sync.dma_start(out=w2h_sb[:], in_=w2h_d.ap())
            w2l_sb = cpool.tile([D, D], bf16)
            nc.sync.dma_start(out=w2l_sb[:], in_=w2l_d.ap())
            b1p_sb = cpool.tile([D, 1], f32)
            nc.sync.dma_start(out=b1p_sb[:], in_=b1p_d.ap())
            b2_sb = cpool.tile([D, 1], f32)
            nc.sync.dma_start(out=b2_sb[:], in_=b2_d.ap())

            # agg stack [hi; hi; lo] built via DMA duplication per supergroup
            aggS = cpool.tile([3 * D, G * P], bf16)  # [96, 12800]
            aggL = cpool.tile([D, G * P], bf16)  # lo scratch [32, 12800]

            CH = 512
            for s in range(NSG):
                edges_t = epool.tile([P, A_ELEMS + B_ELEMS], bf16)
                nc.sync.dma_start(out=edges_t[:], in_=edges_d.ap()[s])
                oh = opool.tile([P, NT, P], bf16)
                nc.vector.tensor_tensor(
                    out=oh[:],
                    in0=iotab[:],
                    in1=idx_all[:, s * NT : (s + 1) * NT].to_broadcast([P, NT, P]),
                    op=mybir.AluOpType.is_equal,
                )
                ps = pspool.tile([D, SG], f32)
                for q in range(QD):
                    nc.tensor.matmul(
                        out=ps[:],
                        lhsT=ident4_sb[:],
                        rhs=edges_t[:, q * SG : (q + 1) * SG],
                        start=(q == 0),
                        stop=False,
                        skip_group_check=True,
                    )
                for jt in range(NT):
                    jj = jt // m_l
                    base = A_ELEMS + jt * 2 * D
                    wcols = slice(jj * P, (jj + 1) * P)
                    nc.tensor.matmul(
                        out=ps[:, wcols],
                        lhsT=edges_t[:, base : base + D],
                        rhs=oh[:, jt, :],
                        start=False,
                        stop=False,
                        skip_group_check=True,
                    )
                    nc.tensor.matmul(
                        out=ps[:, wcols],
                        lhsT=edges_t[:, base + D : base + 2 * D],
                        rhs=oh[:, jt, :],
                        start=False,
                        stop=(jt == NT - 1),
                        skip_group_check=True,
                    )
                sgc = slice(s * SG, (s + 1) * SG)
                nc.scalar.activation(
                    out=aggS[:D, sgc],
                    in_=ps[:],
                    func=mybir.ActivationFunctionType.Copy,
                )
                nc.vector.tensor_tensor(
                    out=aggL[:, sgc],
                    in0=ps[:],
                    in1=aggS[:D, sgc],
                    op=mybir.AluOpType.subtract,
                )
                nc.sync.dma_start(out=aggS[D : 2 * D, sgc], in_=aggS[:D, sgc])
                nc.sync.dma_start(out=aggS[2 * D :, sgc], in_=aggL[:, sgc])

                # MLP chunk for this supergroup's nodes
                n = min(CH, NPC - s * CH)
                cols = slice(s * CH, s * CH + n)
                ph = pmpool.tile([D, CH], f32, tag="ph")
                nc.tensor.matmul(
                    out=ph[:, :n],
                    lhsT=w1n_sb[:],
                    rhs=nodeS_sb[:, cols],
                    start=True,
                    stop=False,
                )
                nc.tensor.matmul(
                    out=ph[:, :n],
                    lhsT=w1g_sb[:],
                    rhs=aggS[:, cols],
                    start=False,
                    stop=True,
                )
                hT = mpool.tile([D, CH], f32, tag="hT")
                nc.scalar.activation(
                    out=hT[:, :n],
                    in_=ph[:, :n],
                    func=mybir.ActivationFunctionType.Relu,
                    bias=b1p_sb[:],
                    scale=1.0,
                )
                hH = mpool.tile([D, CH], bf16, tag="hH")
                nc.scalar.activation(
                    out=hH[:, :n],
                    in_=hT[:, :n],
                    func=mybir.ActivationFunctionType.Copy,
                )
                hL = mpool.tile([D, CH], bf16, tag="hL")
                nc.vector.tensor_tensor(
                    out=hL[:, :n],
                    in0=hT[:, :n],
                    in1=hH[:, :n],
                    op=mybir.AluOpType.subtract,
                )
                po = pmpool.tile([D, CH], f32, tag="po")
                nc.tensor.matmul(
                    out=po[:, :n], lhsT=w2h_sb[:], rhs=hH[:, :n], start=True, stop=False
                )
                nc.tensor.matmul(
                    out=po[:, :n],
                    lhsT=w2l_sb[:],
                    rhs=hH[:, :n],
                    start=False,
                    stop=False,
                )
                nc.tensor.matmul(
                    out=po[:, :n], lhsT=w2h_sb[:], rhs=hL[:, :n], start=False, stop=True
                )
                ot = mpool.tile([D, CH], f32, tag="ot")
                nc.vector.tensor_tensor(
                    out=ot[:, :n],
                    in0=po[:, :n],
                    in1=b2_sb[:].to_broadcast([D, n]),
                    op=mybir.AluOpType.add,
                )
                nc.sync.dma_start(out=outT_d.ap()[:, cols], in_=ot[:, :n])

    nc.finalize()
    _prog_cache[m_l] = nc
    return nc


def kernel(**inputs):
    in_maps, m_l = _host_prep(**inputs)
    nc = _build_program(m_l)
    trace = bool(os.environ.get("KERNEL_TRACE"))
    res = run_bass_kernel_spmd(nc, in_maps, list(range(N_CORES)), trace=trace)
    if trace:
        print(f"HW exec time: {res.exec_time_ns} ns")
        print(f"mean exec time: {res.mean_exec_time_ns} ns")
    out = np.empty((N_NODES, D), dtype=np.float32)
    for c in range(N_CORES):
        out[c * NPC : (c + 1) * NPC] = res.results[c]["outT"].T
    return out



# revision 3
# speedup vs baseline: 2.3589x; 2.3589x over previous
"""Trainium2 Bass kernel for GNN NodeBlock (segment_sum + MLP), 8-core SPMD.

Strategy (degree-sorted node sharding, fp16 single-copy, dense-only):
  - Sort nodes by degree (descending) on the host and regroup into 200
    supergroups of 500 nodes. Supergroup g goes to core g%8 at position
    g//8, so every position k holds 8 groups of near-identical max degree
    K̄_k — the SPMD program pads each node to K̄_k edge slots with almost
    no waste and needs no overflow path at all.
  - Edges ship once, as fp16 (rel err ~2^-11, well inside the 2e-2 gate).
    Slots are packed 4 per matmul: a [128, 500] tile whose partition axis
    is (slot-in-quad, feature) streams through a stationary [I32;I32;I32;
    I32], accumulating the feature-major aggregate psum[32, 500] directly.
    Remainder slots (K̄ mod 4) live in a compact [32r, 500] side tensor.
  - MLP: rhs = [node; agg] stacked [64, 500] fp16, W1 [64,32] fp16 single
    matmul (globals folded into b1 on host), Relu+bias on ACT, W2 [32,32]
    fp16 matmul, bias+cast drain on DVE. Agg psum drain on Pool. The MLP
    runs one supergroup behind the aggregation so cross-engine drains stay
    off the PE critical path.
  - No collectives: cores own disjoint node sets; host permutes/gathers.
"""

import os

import numpy as np

import concourse.bacc as bacc
import concourse.bass as bass
import concourse.mybir as mybir
import concourse.tile as tile
from concourse.bass_utils import run_bass_kernel_spmd

N_NODES = 100000
N_CORES = 8
NPC = N_NODES // N_CORES  # 12500 nodes per core
P = 128
SG = 500  # nodes per supergroup
NPOS = NPC // SG  # 25 supergroup positions per core
NG = N_CORES * NPOS  # 200 groups
D = 32

_prog_cache = {}


def _host_prep(node_attr, edge_index, edge_attr, global_attr, W1, b1, W2, b2):
    E = edge_attr.shape[0]
    r = np.ascontiguousarray(edge_index[1]).astype(np.int64)

    deg = np.bincount(r, minlength=N_NODES)
    perm = np.argsort(-deg, kind="stable")  # new id -> old id, degree desc
    newid = np.empty(N_NODES, dtype=np.int64)
    newid[perm] = np.arange(N_NODES, dtype=np.int64)
    degs = deg[perm]  # degree by new id (non-increasing)

    rn = newid[r]  # receiver in new ids
    grp = rn // SG  # 0..199
    k_pos = grp // N_CORES  # supergroup position 0..24
    core = grp % N_CORES
    n_in = rn % SG  # column within supergroup

    # rank of each edge within its node
    order = np.argsort(rn, kind="stable")
    starts = np.zeros(N_NODES, dtype=np.int64)
    np.cumsum(degs[:-1], out=starts[1:])
    rank = np.empty(E, dtype=np.int64)
    rank[order] = np.arange(E, dtype=np.int64) - starts[rn[order]]

    # per-position slot count = max degree among its 8*SG nodes
    Kbar = np.maximum(degs.reshape(NPOS, N_CORES * SG).max(axis=1), 1)
    Q4 = Kbar // 4  # full quad matmuls
    R = Kbar % 4  # remainder slots
    q4_off = np.zeros(NPOS + 1, dtype=np.int64)
    np.cumsum(Q4 * SG, out=q4_off[1:])
    rem_off = np.zeros(NPOS + 1, dtype=np.int64)
    np.cumsum((R > 0) * SG, out=rem_off[1:])
    q4_total = int(q4_off[-1])
    rem_total = max(int(rem_off[-1]), SG)

    ea16 = np.ascontiguousarray(edge_attr, dtype=np.float32).astype(np.float16)

    q_e = rank // 4
    j_e = rank % 4
    in_q4 = q_e < Q4[k_pos]

    A = np.zeros((N_CORES, 4, D, q4_total), dtype=np.float16)
    m = in_q4
    A[core[m], j_e[m], :, q4_off[k_pos[m]] + q_e[m] * SG + n_in[m]] = ea16[m]
    A = A.reshape(N_CORES, P, q4_total)

    REM = np.zeros((N_CORES, 3, D, rem_total), dtype=np.float16)
    m = ~in_q4
    jr = rank[m] - 4 * Q4[k_pos[m]]
    REM[core[m], jr, :, rem_off[k_pos[m]] + n_in[m]] = ea16[m]
    REM = REM.reshape(N_CORES, 3 * D, rem_total)

    # node features, permuted + arranged per core/position, transposed
    nodeP = np.asarray(node_attr, dtype=np.float32)[perm].astype(np.float16)
    nodeT = np.ascontiguousarray(
        nodeP.reshape(NG, SG, D).reshape(NPOS, N_CORES, SG, D).transpose(1, 3, 0, 2)
    ).reshape(N_CORES, D, NPC)

    g0 = np.asarray(global_attr, dtype=np.float32).reshape(1, D)
    W1 = np.asarray(W1, dtype=np.float32)
    b1p = (np.asarray(b1, dtype=np.float32) + (g0 @ W1[2 * D :]).reshape(-1)).reshape(
        D, 1
    )
    w1 = np.ascontiguousarray(W1[: 2 * D]).astype(np.float16)  # [64, 32]
    w2 = np.ascontiguousarray(np.asarray(W2, dtype=np.float32)).astype(np.float16)
    b2p = np.asarray(b2, dtype=np.float32).reshape(D, 1)

    ident4 = np.ascontiguousarray(np.tile(np.eye(D, dtype=np.float16), (4, 1)))

    in_maps = []
    for c in range(N_CORES):
        in_maps.append(
            {
                "edges": A[c],
                "rem": REM[c],
                "ident4": ident4,
                "nodeT": nodeT[c],
                "w1": w1,
                "w2": w2,
                "b1p": b1p,
                "b2p": b2p,
            }
        )
    return in_maps, tuple(int(q) for q in Q4), tuple(int(x) for x in R), perm


def _build_program(Q4, R):
    key = (Q4, R)
    if key in _prog_cache:
        return _prog_cache[key]

    f16 = mybir.dt.float16
    f32 = mybir.dt.float32
    nc = bacc.Bacc(
        "TRN2", target_bir_lowering=False, debug=False, num_devices=N_CORES
    )

    q4_off = [0]
    for q in Q4:
        q4_off.append(q4_off[-1] + q * SG)
    rem_off = [0]
    for r in R:
        rem_off.append(rem_off[-1] + (SG if r > 0 else 0))
    q4_total = q4_off[-1]
    rem_total = max(rem_off[-1], SG)

    edges_d = nc.dram_tensor("edges", [P, q4_total], f16, kind="ExternalInput")
    rem_d = nc.dram_tensor("rem", [3 * D, rem_total], f16, kind="ExternalInput")
    ident4_d = nc.dram_tensor("ident4", [P, D], f16, kind="ExternalInput")
    nodeT_d = nc.dram_tensor("nodeT", [D, NPC], f16, kind="ExternalInput")
    w1_d = nc.dram_tensor("w1", [2 * D, D], f16, kind="ExternalInput")
    w2_d = nc.dram_tensor("w2", [D, D], f16, kind="ExternalInput")
    b1p_d = nc.dram_tensor("b1p", [D, 1], f32, kind="ExternalInput")
    b2p_d = nc.dram_tensor("b2p", [D, 1], f32, kind="ExternalInput")
    outT_d = nc.dram_tensor("outT", [D, NPC], f16, kind="ExternalOutput")

    with tile.TileContext(nc) as tc:
        with (
            tc.tile_pool(name="const", bufs=1) as cpool,
            tc.tile_pool(name="edges", bufs=3) as epool,
            tc.tile_pool(name="rem", bufs=3) as rpool,
            tc.tile_pool(name="mlp", bufs=3) as mpool,
            tc.tile_pool(name="psA", bufs=3, space="PSUM") as pspool,
            tc.tile_pool(name="ps1", bufs=2, space="PSUM") as p1pool,
            tc.tile_pool(name="ps2", bufs=2, space="PSUM") as p2pool,
        ):
            ident4_sb = cpool.tile([P, D], f16)
            nc.sync.dma_start(out=ident4_sb[:], in_=ident4_d.ap())
            w1_sb = cpool.tile([2 * D, D], f16)
            nc.sync.dma_start(out=w1_sb[:], in_=w1_d.ap())
            w2_sb = cpool.tile([D, D], f16)
            nc.sync.dma_start(out=w2_sb[:], in_=w2_d.ap())
            b1p_sb = cpool.tile([D, 1], f32)
            nc.sync.dma_start(out=b1p_sb[:], in_=b1p_d.ap())
            b2p_sb = cpool.tile([D, 1], f32)
            nc.sync.dma_start(out=b2p_sb[:], in_=b2p_d.ap())

            # [node(0:32); agg(32:64)] stacked MLP input, and the fp16 output
            nodeAgg = cpool.tile([2 * D, NPC], f16)
            nc.sync.dma_start(out=nodeAgg[:D, :], in_=nodeT_d.ap())
            outb = cpool.tile([D, NPC], f16)

            # aggregation psum per position; MLP pipelined one position behind
            ps_hist = {}
            ph_hist = {}
            po_hist = {}

            def mlp_front(s):
                cols = slice(s * SG, (s + 1) * SG)
                ph = p1pool.tile([D, SG], f32, tag="ph")
                nc.tensor.matmul(
                    out=ph[:],
                    lhsT=w1_sb[:],
                    rhs=nodeAgg[:, cols],
                    start=True,
                    stop=True,
                )
                hH = mpool.tile([D, SG], f16, tag="hH")
                nc.scalar.activation(
                    out=hH[:],
                    in_=ph[:],
                    func=mybir.ActivationFunctionType.Relu,
                    bias=b1p_sb[:],
                    scale=1.0,
                )
                ph_hist[s] = hH

            def mlp_back(s):
                cols = slice(s * SG, (s + 1) * SG)
                hH = ph_hist.pop(s)
                po = p2pool.tile([D, SG], f32, tag="po")
                nc.tensor.matmul(
                    out=po[:], lhsT=w2_sb[:], rhs=hH[:], start=True, stop=True
                )
                nc.vector.tensor_tensor(
                    out=outb[:, cols],
                    in0=po[:],
                    in1=b2p_sb[:].to_broadcast([D, SG]),
                    op=mybir.AluOpType.add,
                )

            for s in range(NPOS):
                q4 = Q4[s]
                r = R[s]
                cols = slice(s * SG, (s + 1) * SG)

                et = None
                if q4 > 0:
                    et = epool.tile([P, Q4[0] * SG], f16, tag="et")
                    nc.sync.dma_start(
                        out=et[:, : q4 * SG],
                        in_=edges_d.ap()[:, q4_off[s] : q4_off[s] + q4 * SG],
                    )
                rt = None
                if r > 0:
                    rt = rpool.tile([3 * D, SG], f16, tag="rt")
                    nc.sync.dma_start(
                        out=rt[: r * D, :],
                        in_=rem_d.ap()[: r * D, rem_off[s] : rem_off[s] + SG],
                    )

                ps = pspool.tile([D, SG], f32, tag="ps")
                for q in range(q4):
                    nc.tensor.matmul(
                        out=ps[:],
                        lhsT=ident4_sb[:],
                        rhs=et[:, q * SG : (q + 1) * SG],
                        start=(q == 0),
                        stop=(q == q4 - 1 and r == 0),
                        skip_group_check=True,
                    )
                if r > 0:
                    nc.tensor.matmul(
                        out=ps[:],
                        lhsT=ident4_sb[: r * D, :],
                        rhs=rt[: r * D, :],
                        start=(q4 == 0),
                        stop=True,
                        skip_group_check=True,
                    )
                # drain agg into the stacked MLP input (DVE; Pool can't read PSUM)
                nc.vector.tensor_copy(out=nodeAgg[D:, cols], in_=ps[:])

                # MLP pipelined: front of s-1, back of s-2
                if s >= 1:
                    mlp_front(s - 1)
                if s >= 2:
                    mlp_back(s - 2)

            mlp_front(NPOS - 1)
            mlp_back(NPOS - 2)
            mlp_back(NPOS - 1)

            nc.sync.dma_start(out=outT_d.ap(), in_=outb[:])

    nc.finalize()
    _prog_cache[key] = nc
    return nc


def kernel(**inputs):
    in_maps, Q4, R, perm = _host_prep(**inputs)
    nc = _build_program(Q4, R)
    trace = bool(os.environ.get("KERNEL_TRACE"))
    res = run_bass_kernel_spmd(nc, in_maps, list(range(N_CORES)), trace=trace)
    if trace:
        print(f"HW exec time: {res.exec_time_ns} ns")
        print(f"mean exec time: {res.mean_exec_time_ns} ns")
    out_all = np.empty((NPOS, N_CORES, SG, D), dtype=np.float32)
    for c in range(N_CORES):
        out_all[:, c] = (
            res.results[c]["outT"].astype(np.float32).T.reshape(NPOS, SG, D)
        )
    out = np.empty((N_NODES, D), dtype=np.float32)
    out[perm] = out_all.reshape(N_NODES, D)
    return out


# revision 6
# speedup vs baseline: 3.0261x; 1.2828x over previous
"""Trainium2 Bass kernel for GNN NodeBlock (segment_sum + MLP), 8-core SPMD.

Strategy (degree-sorted node sharding, fp16 single-copy, dense-only):
  - Sort nodes by degree (ascending) on the host and regroup into 200
    supergroups of 500 nodes. Supergroup g goes to core g%8 at position
    g//8, so every position k holds 8 groups of near-identical max degree
    K̄_k — the SPMD program pads each node to K̄_k edge slots (rounded up
    to a multiple of 4) with little waste and needs no overflow path at
    all. Ascending order puts the smallest tiles first, so the PE starts
    within ~2µs of kernel start.
  - Edges ship once, as fp16 (rel err ~2^-11, well inside the 2e-2 gate).
    Slots are packed 4 per quad: a [128, 500] block whose partition axis
    is (slot-in-quad, feature) streams through a stationary [I32;I32;I32;
    I32], accumulating the feature-major aggregate psum[32, 500] directly.
  - MLP: rhs = [node; agg] stacked [64, 500] fp16, W1 [64,32] fp16 single
    matmul (globals folded into b1 on host), Relu+bias on ACT, W2 [32,32]
    fp16 matmul, bias+cast drain on DVE. Agg psum drain on DVE. The MLP
    runs one supergroup behind the aggregation so cross-engine drains stay
    off the PE critical path. Output flushes to HBM in chunks on the ACT
    queue; edge tiles prefetch 6 deep on the SP queue.
  - No collectives: cores own disjoint node sets; host permutes/gathers.
"""

import os

import numpy as np

import concourse.bacc as bacc
import concourse.bass as bass
import concourse.mybir as mybir
import concourse.tile as tile
from concourse.bass_utils import run_bass_kernel_spmd

N_NODES = 100000
N_CORES = 8
NPC = N_NODES // N_CORES  # 12500 nodes per core
P = 128
SG = 500  # nodes per supergroup
NPOS = NPC // SG  # 25 supergroup positions per core
NG = N_CORES * NPOS  # 200 groups
D = 32

_prog_cache = {}


def _host_prep(node_attr, edge_index, edge_attr, global_attr, W1, b1, W2, b2):
    E = edge_attr.shape[0]
    r = np.ascontiguousarray(edge_index[1]).astype(np.int64)

    deg = np.bincount(r, minlength=N_NODES)
    perm = np.argsort(deg, kind="stable")  # new id -> old id, degree asc
    newid = np.empty(N_NODES, dtype=np.int64)
    newid[perm] = np.arange(N_NODES, dtype=np.int64)
    degs = deg[perm]  # degree by new id (non-decreasing)

    rn = newid[r]  # receiver in new ids
    grp = rn // SG  # 0..199
    k_pos = grp // N_CORES  # supergroup position 0..24
    core = grp % N_CORES
    n_in = rn % SG  # column within supergroup

    # rank of each edge within its node
    order = np.argsort(rn, kind="stable")
    starts = np.zeros(N_NODES, dtype=np.int64)
    np.cumsum(degs[:-1], out=starts[1:])
    rank = np.empty(E, dtype=np.int64)
    rank[order] = np.arange(E, dtype=np.int64) - starts[rn[order]]

    # per-position quad count: max degree among its 8*SG nodes, / 4 rounded up
    Kbar = np.maximum(degs.reshape(NPOS, N_CORES * SG).max(axis=1), 1)
    Q = -(-Kbar // 4)
    q_off = np.zeros(NPOS + 1, dtype=np.int64)
    np.cumsum(Q * SG, out=q_off[1:])
    q_total = int(q_off[-1])

    ea16 = np.ascontiguousarray(edge_attr, dtype=np.float32).astype(np.float16)

    A = np.zeros((N_CORES, 4, D, q_total), dtype=np.float16)
    A[core, rank % 4, :, q_off[k_pos] + (rank // 4) * SG + n_in] = ea16
    A = A.reshape(N_CORES, P, q_total)

    # node features, permuted + arranged per core/position, transposed
    nodeP = np.asarray(node_attr, dtype=np.float32)[perm].astype(np.float16)
    nodeT = np.ascontiguousarray(
        nodeP.reshape(NG, SG, D).reshape(NPOS, N_CORES, SG, D).transpose(1, 3, 0, 2)
    ).reshape(N_CORES, D, NPC)

    g0 = np.asarray(global_attr, dtype=np.float32).reshape(1, D)
    W1 = np.asarray(W1, dtype=np.float32)
    b1p = (np.asarray(b1, dtype=np.float32) + (g0 @ W1[2 * D :]).reshape(-1)).reshape(
        D, 1
    )
    w1 = np.ascontiguousarray(W1[: 2 * D]).astype(np.float16)  # [64, 32]
    w2 = np.ascontiguousarray(np.asarray(W2, dtype=np.float32)).astype(np.float16)
    b2p = np.asarray(b2, dtype=np.float32).reshape(D, 1)

    ident4 = np.ascontiguousarray(np.tile(np.eye(D, dtype=np.float16), (4, 1)))

    in_maps = []
    for c in range(N_CORES):
        in_maps.append(
            {
                "edges": A[c],
                "ident4": ident4,
                "nodeT": nodeT[c],
                "w1": w1,
                "w2": w2,
                "b1p": b1p,
                "b2p": b2p,
            }
        )
    return in_maps, tuple(int(q) for q in Q), perm


def _build_program(Q):
    if Q in _prog_cache:
        return _prog_cache[Q]

    f16 = mybir.dt.float16
    f32 = mybir.dt.float32
    nc = bacc.Bacc(
        "TRN2", target_bir_lowering=False, debug=False, num_devices=N_CORES
    )

    q_off = [0]
    for q in Q:
        q_off.append(q_off[-1] + q * SG)
    q_total = q_off[-1]
    Qmax = max(Q)

    edges_d = nc.dram_tensor("edges", [P, q_total], f16, kind="ExternalInput")
    ident4_d = nc.dram_tensor("ident4", [P, D], f16, kind="ExternalInput")
    nodeT_d = nc.dram_tensor("nodeT", [D, NPC], f16, kind="ExternalInput")
    w1_d = nc.dram_tensor("w1", [2 * D, D], f16, kind="ExternalInput")
    w2_d = nc.dram_tensor("w2", [D, D], f16, kind="ExternalInput")
    b1p_d = nc.dram_tensor("b1p", [D, 1], f32, kind="ExternalInput")
    b2p_d = nc.dram_tensor("b2p", [D, 1], f32, kind="ExternalInput")
    outT_d = nc.dram_tensor("outT", [D, NPC], f16, kind="ExternalOutput")

    with tile.TileContext(nc) as tc:
        with (
            tc.tile_pool(name="const", bufs=1) as cpool,
            tc.tile_pool(name="edges", bufs=6) as epool,
            tc.tile_pool(name="mlp", bufs=3) as mpool,
            tc.tile_pool(name="psA", bufs=3, space="PSUM") as pspool,
            tc.tile_pool(name="ps1", bufs=2, space="PSUM") as p1pool,
            tc.tile_pool(name="ps2", bufs=2, space="PSUM") as p2pool,
        ):
            # first edge tile on the SP queue before anything else
            et0 = epool.tile([P, Qmax * SG], f16, tag="et")
            nc.sync.dma_start(
                out=et0[:, : Q[0] * SG], in_=edges_d.ap()[:, q_off[0] : q_off[1]]
            )

            # consts on the ACT queue (parallel with the edge stream)
            ident4_sb = cpool.tile([P, D], f16)
            nc.scalar.dma_start(out=ident4_sb[:], in_=ident4_d.ap())
            w1_sb = cpool.tile([2 * D, D], f16)
            nc.scalar.dma_start(out=w1_sb[:], in_=w1_d.ap())
            w2_sb = cpool.tile([D, D], f16)
            nc.scalar.dma_start(out=w2_sb[:], in_=w2_d.ap())
            b1p_sb = cpool.tile([D, 1], f32)
            nc.scalar.dma_start(out=b1p_sb[:], in_=b1p_d.ap())
            b2p_sb = cpool.tile([D, 1], f32)
            nc.scalar.dma_start(out=b2p_sb[:], in_=b2p_d.ap())

            # [node(0:32); agg(32:64)] stacked MLP input, and the fp16 output
            nodeAgg = cpool.tile([2 * D, NPC], f16)
            nc.scalar.dma_start(out=nodeAgg[:D, :], in_=nodeT_d.ap())
            outb = cpool.tile([D, NPC], f16)

            hH_hist = {}

            def mlp_front(s):
                cols = slice(s * SG, (s + 1) * SG)
                ph = p1pool.tile([D, SG], f32, tag="ph")
                nc.tensor.matmul(
                    out=ph[:],
                    lhsT=w1_sb[:],
                    rhs=nodeAgg[:, cols],
                    start=True,
                    stop=True,
                )
                hH = mpool.tile([D, SG], f16, tag="hH")
                nc.scalar.activation(
                    out=hH[:],
                    in_=ph[:],
                    func=mybir.ActivationFunctionType.Relu,
                    bias=b1p_sb[:],
                    scale=1.0,
                )
                hH_hist[s] = hH

            def mlp_back(s):
                cols = slice(s * SG, (s + 1) * SG)
                hH = hH_hist.pop(s)
                po = p2pool.tile([D, SG], f32, tag="po")
                nc.tensor.matmul(
                    out=po[:], lhsT=w2_sb[:], rhs=hH[:], start=True, stop=True
                )
                nc.vector.tensor_tensor(
                    out=outb[:, cols],
                    in0=po[:],
                    in1=b2p_sb[:].to_broadcast([D, SG]),
                    op=mybir.AluOpType.add,
                )

            out_flushed = 0

            def flush_out(upto):
                nonlocal out_flushed
                if upto > out_flushed:
                    cols = slice(out_flushed * SG, upto * SG)
                    nc.scalar.dma_start(out=outT_d.ap()[:, cols], in_=outb[:, cols])
                    out_flushed = upto

            for s in range(NPOS):
                q = Q[s]
                cols = slice(s * SG, (s + 1) * SG)

                if s == 0:
                    et = et0
                else:
                    et = epool.tile([P, Qmax * SG], f16, tag="et")
                    nc.sync.dma_start(
                        out=et[:, : q * SG],
                        in_=edges_d.ap()[:, q_off[s] : q_off[s] + q * SG],
                    )

                ps = pspool.tile([D, SG], f32, tag="ps")
                for j in range(q):
                    nc.tensor.matmul(
                        out=ps[:],
                        lhsT=ident4_sb[:],
                        rhs=et[:, j * SG : (j + 1) * SG],
                        start=(j == 0),
                        stop=(j == q - 1),
                        skip_group_check=True,
                    )
                # drain agg into the stacked MLP input (DVE; Pool can't read PSUM)
                nc.vector.tensor_copy(out=nodeAgg[D:, cols], in_=ps[:])

                # MLP pipelined: front of s-1, back of s-2
                if s >= 1:
                    mlp_front(s - 1)
                if s >= 2:
                    mlp_back(s - 2)
                    if (s - 2) % 5 == 4:
                        flush_out(s - 2 + 1)

            mlp_front(NPOS - 1)
            mlp_back(NPOS - 2)
            mlp_back(NPOS - 1)
            flush_out(NPOS)

    nc.finalize()
    _prog_cache[Q] = nc
    return nc


def kernel(**inputs):
    in_maps, Q, perm = _host_prep(**inputs)
    nc = _build_program(Q)
    trace = bool(os.environ.get("KERNEL_TRACE"))
    res = run_bass_kernel_spmd(nc, in_maps, list(range(N_CORES)), trace=trace)
    if trace:
        print(f"HW exec time: {res.exec_time_ns} ns")
        print(f"mean exec time: {res.mean_exec_time_ns} ns")
    out_all = np.empty((NPOS, N_CORES, SG, D), dtype=np.float32)
    for c in range(N_CORES):
        out_all[:, c] = (
            res.results[c]["outT"].astype(np.float32).T.reshape(NPOS, SG, D)
        )
    out = np.empty((N_NODES, D), dtype=np.float32)
    out[perm] = out_all.reshape(N_NODES, D)
    return out
